# revision 1
# baseline (speedup 1.0000x reference)
"""Trainium2 Bass kernel for one cyclical-Langevin (Gibbs-with-gradients) MH step.

Full inputs -> full outputs; internally data-parallel across 8 NeuronCores
(1024 chains per core).

Math (per chain row x in {0,1}^784, y := 1-2x in {+-1}^784):
    S  = W + W^T                 gx_full = x@S + b = b2 + y@S2,
    S2 = -S/2, b2 = b + S@1/2
    d  = y * gx_full             (= diff function)
    ind = logit(rr) + term2 < d  (== rr < sigmoid(d - term2))
    y' = flip sign of y where ind
    w  = y' * (b2 + y'@S2)
    la = sum_j [ sp(zf) - sp(zr) + ind*(w-d) ] - .25*sum w + .25*sum d
         + 0.25*[sum_j gd - sum_j gx + b.(y-y')]
    accept a = la > log(rr_acc); x_new = a ? x^flips : x

The matmul runs against an augmented matrix Aaug[785,785]:
    Aaug[:784,:784] = S2 ; Aaug[784,:784] = b2 (ones-row bias)
    Aaug[:784,784]  = 0.25*(rowsum(S2) - b) ; Aaug[784,784] = 0.25*sum(b2)
so psum = yaug@Aaug gives gx_full in cols 0:784 and the 0.25*(sum_j g - b.y)
m_term piece in col 784.

Precision: PE fp32 matmul is 4x slow and float32r needs explicit lossy
rounding, so Aaug is split hi/lo into fp16: Abig = [fp16(Aaug);
fp16((Aaug-hi)*2^12)] with the stationary side [yT; yT*2^-12]. Products are
exact (stationary is +-1 or +-2^-12), accumulation is fp32 in PSUM; A is
represented to ~2^-22 relative => f32-class decision boundaries.
"""

import math
from contextlib import ExitStack

import numpy as np
import ml_dtypes

import concourse.bass as bass
import concourse.mybir as mybir
import concourse.tile as tile
from concourse import bacc
from concourse.bass_utils import run_bass_kernel_spmd
from concourse.masks import make_identity

MEAN_STEP = 0.2
NUM_CYCLES = 5
NUM_ITERS = 500
ITER_PER_CYCLE = math.ceil(NUM_ITERS / NUM_CYCLES)

B, D = 8192, 784
NCORES = 8
BC = B // NCORES          # chains per core
DA = D + 1                # augmented contraction size (ones row)
KB = 2 * DA               # hi/lo stacked contraction size (1570)
NK = 14                   # K chunks: (6x112 + 113) x 2
KC = 112
LOSCALE = 2.0 ** -12

f32 = mybir.dt.float32
fp16 = mybir.dt.float16
bf16 = mybir.dt.bfloat16
u16 = mybir.dt.uint16
Alu = mybir.AluOpType
Act = mybir.ActivationFunctionType

# psum layout: [128, 1024] fp32 = 2 banks. half0 = A cols 0:393 at psum cols
# 0:393 (bank 0); half1 = A cols 393:785 at psum cols 512:904 (bank 1).
H0 = 393                  # A-columns in half 0
H1 = DA - H0              # 392, half 1 (includes aug col at its end)
POFF = 512                # psum col offset of half 1


def _kchunk(c):
    # rows of Abig / yTbig covered by K-chunk c (c in 0..13)
    base = 0 if c < 7 else DA
    cc = c % 7
    lo = base + cc * KC
    hi = base + (DA if cc == 6 else (cc + 1) * KC)
    return lo, hi


def _pin_act_tables():
    """Force Exp and Ln onto the one table that holds both
    (natural_log_exp_and_others) — the default greedy pick alternates between
    exp_and_others and natural_log, reloading the ACT table twice per tile
    (~1.3us per reload). Table ids must keep their act_info.json positions,
    so remove Exp/Ln from the other tables instead of filtering the list."""
    if getattr(bacc, "_act_tables_pinned", False):
        return
    orig = bacc.get_activation_tables

    def pinned(arch):
        out = {}
        for name, funcs in orig(arch).items():
            if name != "natural_log_exp_and_others":
                funcs = funcs - {Act.Exp, Act.Ln}
            out[name] = funcs
        return out

    bacc.get_activation_tables = pinned
    bacc._act_tables_pinned = True


def build_program(bc=BC, repeat=1, loop_repeat=1):
    """Build the (single-core SPMD) Bass program for bc chains.

    repeat>1 statically re-runs the tile loop; loop_repeat>1 wraps it in a
    device-side For_i — both for differential wall-clock timing (slope =
    per-iteration HW time)."""
    _pin_act_tables()
    nt = bc // 128
    nc = bacc.Bacc(target_bir_lowering=False, debug=False)

    i_A = nc.dram_tensor("Abig", [KB, DA], fp16, kind="ExternalInput").ap()
    i_yT = nc.dram_tensor("yTaug", [DA, bc], fp16, kind="ExternalInput").ap()
    i_y = nc.dram_tensor("y", [bc, D], bf16, kind="ExternalInput").ap()
    i_lr2 = nc.dram_tensor("lr2", [bc, D], f32, kind="ExternalInput").ap()
    i_lacc = nc.dram_tensor("lacc", [128, nt], f32, kind="ExternalInput").ap()
    i_t2n = nc.dram_tensor("t2neg", [128, 1], f32, kind="ExternalInput").ap()
    o_y = nc.dram_tensor("out_y", [bc, D], bf16, kind="ExternalOutput").ap()

    with tile.TileContext(nc) as tc, ExitStack() as ctx:
        cpool = ctx.enter_context(tc.tile_pool(name="const", bufs=1))
        wpool = ctx.enter_context(tc.tile_pool(name="work", bufs=3))
        colp = ctx.enter_context(tc.tile_pool(name="cols", bufs=3))
        pmm = ctx.enter_context(tc.tile_pool(name="pmm", bufs=3, space="PSUM"))
        ptp = ctx.enter_context(tc.tile_pool(name="ptp", bufs=2, space="PSUM"))

        # ---- persistent constants ----
        A_sb = []
        yT_sb = []
        for c in range(NK):
            lo, hi = _kchunk(c)
            at = cpool.tile([hi - lo, DA], fp16, tag=f"A{c}")
            nc.sync.dma_start(at[:], i_A[lo:hi, :])
            A_sb.append(at)
            yt = cpool.tile([hi - lo, bc], fp16, tag=f"yT{c}")
            if c < NK // 2:
                nc.sync.dma_start(yt[:], i_yT[lo:hi, :])
            else:
                # lo-half stationary = hi-half * 2^-12, derived on the (idle
                # at startup) ACT engine instead of 1.6MB more DMA
                nc.scalar.activation(yt[:], yT_sb[c - NK // 2][:], Act.Copy,
                                     scale=LOSCALE)
            yT_sb.append(yt)
        ident = cpool.tile([128, 128], bf16, tag="ident")
        make_identity(nc, ident[:])
        t2n = cpool.tile([128, 1], f32, tag="t2n")
        nc.sync.dma_start(t2n[:], i_t2n[:])
        sh15 = cpool.tile([128, 1], u16, tag="sh15")
        nc.vector.memset(sh15[:], 15)
        lacc = cpool.tile([128, nt], f32, tag="lacc")
        nc.sync.dma_start(lacc[:], i_lacc[:])

        import contextlib
        loop_cm = (tc.For_i(0, loop_repeat, 1) if loop_repeat > 1
                   else contextlib.nullcontext())
        with loop_cm:
            _tile_body(nc, tc, nt, repeat, A_sb, yT_sb, ident, t2n, sh15, lacc,
                       i_y, i_lr2, o_y, wpool, colp, pmm, ptp)

    nc.compile()
    return nc


def _tile_body(nc, tc, nt, repeat, A_sb, yT_sb, ident, t2n, sh15, lacc,
               i_y, i_lr2, o_y, wpool, colp, pmm, ptp):
        def emit_mm1(t, part=None):
            rs = t * 128
            if part is None:
                p1 = pmm.tile([128, 1024], f32, tag="mm", name=f"p1_{t}")
                lo_c, hi_c = 0, NK
            else:
                p1, lo_c, hi_c = part
            for c in range(lo_c, hi_c):
                lhsT = yT_sb[c][:, rs:rs + 128]
                nc.tensor.matmul(p1[:, 0:H0], lhsT, A_sb[c][:, 0:H0],
                                 start=(c == 0), stop=(c == NK - 1))
                nc.tensor.matmul(p1[:, POFF:POFF + H1], lhsT, A_sb[c][:, H0:DA],
                                 start=(c == 0), stop=(c == NK - 1))
            return p1

        def emit_tail(rs, t, spf_c, spr_c, iw_c, id_c, accw0, accw1,
                      accd0, accd1, gd4, gx4, y_t, yp):
            # deferred one tile: the in-order DVE would otherwise stall
            # here waiting for ACT's softplus accums while the next
            # tile's d/ind/y' chain is already runnable.
                # ---- log-accept assembly (per-partition column math) ----
                u1 = colp.tile([128, 1], f32)
                nc.vector.tensor_scalar(u1[:], accw0[:], accw1[:], -0.25,
                                        Alu.add, Alu.mult)
                u2 = colp.tile([128, 1], f32)
                nc.vector.tensor_scalar(u2[:], accd0[:], accd1[:], 0.25,
                                        Alu.add, Alu.mult)
                a1 = colp.tile([128, 1], f32)
                nc.vector.tensor_scalar(a1[:], spf_c[:], spr_c[:], iw_c[:],
                                        Alu.subtract, Alu.add)
                a2 = colp.tile([128, 1], f32)
                nc.vector.tensor_scalar(a2[:], u1[:], u2[:], a1[:],
                                        Alu.add, Alu.add)
                a4 = colp.tile([128, 1], f32)
                nc.vector.tensor_scalar(a4[:], gd4[:], id_c[:], a2[:],
                                        Alu.subtract, Alu.add)
                la = colp.tile([128, 1], f32)
                nc.vector.tensor_scalar(la[:], gx4[:], -1.0, a4[:],
                                        Alu.mult, Alu.add)
                amask = colp.tile([128, 1], u16)
                nc.vector.tensor_scalar(amask[:], la[:], lacc[:, t:t + 1], None,
                                        Alu.is_gt)

                # ---- select + store ----
                ynew = wpool.tile([128, D], bf16)
                nc.gpsimd.tensor_copy(ynew[:], y_t[:])
                nc.vector.copy_predicated(ynew[:], amask[:].broadcast_to((128, D)),
                                          yp[:])
                nc.sync.dma_start(o_y[rs:rs + 128, :], ynew[:])

        # software pipeline: mm1 for tile t+1 is emitted (and runs on PE)
        # while tile t's elementwise chain + evictions proceed, so the
        # in-order PE never idles waiting for tile t's mm2 inputs.
        niter = nt * repeat
        p1_next = emit_mm1(0)
        for t in range(niter):
            is_last = t == niter - 1
            t = t % nt
            rs = t * 128
            p1 = p1_next

            y_t = wpool.tile([128, D], bf16)
            nc.sync.dma_start(y_t[:], i_y[rs:rs + 128, :])
            lr2_t = wpool.tile([128, D], f32)
            nc.sync.dma_start(lr2_t[:], i_lr2[rs:rs + 128, :])

            # ---- d = y * gx_full ; accd = sum_j d ; gx4 (aug col) ----
            # (tensor_tensor_reduce crashes on HW; scalar_tensor_tensor with
            # accum_out is the working equivalent, one accumulator per half)
            d_t = wpool.tile([128, D], f32)
            accd0 = colp.tile([128, 1], f32)
            accd1 = colp.tile([128, 1], f32)
            nc.vector.scalar_tensor_tensor(
                d_t[:, 0:H0], p1[:, 0:H0], 1.0, y_t[:, 0:H0],
                Alu.mult, Alu.mult, accum_out=accd0[:])
            nc.vector.scalar_tensor_tensor(
                d_t[:, H0:D], p1[:, POFF:POFF + (D - H0)], 1.0, y_t[:, H0:D],
                Alu.mult, Alu.mult, accum_out=accd1[:])
            gx4 = colp.tile([128, 1], f32)
            nc.vector.tensor_copy(gx4[:], p1[:, POFF + H1 - 1:POFF + H1])

            # ---- flips ----
            ind = wpool.tile([128, D], u16)
            nc.vector.tensor_tensor(ind[:], lr2_t[:], d_t[:], Alu.is_lt)
            yp = wpool.tile([128, D], bf16)
            nc.vector.scalar_tensor_tensor(
                yp[:].bitcast(u16), ind[:], sh15[:], y_t[:].bitcast(u16),
                Alu.logical_shift_left, Alu.bitwise_xor)

            # first half of next tile's mm1 ahead of the transposes, the
            # second half after them: the PE streams mm1 while ACT evicts
            # this tile's y'T, so mm2 finds its stationary tiles ready.
            if not is_last:
                tn = (t + 1) % nt
                p1_next = pmm.tile([128, 1024], f32, tag="mm", name=f"p1_{tn}")
                emit_mm1(tn, part=(p1_next, 0, NK // 2))

            # ---- transpose y' -> y'T chunks (PE), evict hi+lo to SBUF ----
            # one [112, 896] bf16 tile = 1792B/partition = a single PSUM bank
            tpab = ptp.tile([112, 896], bf16, tag="tp")
            tpa = tpab[:, 0:512]
            tpb = tpab[:, 512:896]
            for c in range(7):
                nc.tensor.transpose(tpab[:, c * 128:(c + 1) * 128],
                                    yp[:, c * KC:(c + 1) * KC], ident[:])
            if not is_last:
                emit_mm1(tn, part=(p1_next, NK // 2, NK))

            # ones row for the augmented contraction lives at partition 112;
            # engines can only start at partition 0/32/64/96, so memset rows
            # 96:128 first and let the transpose evict overwrite rows 0:112.
            ta = wpool.tile([112, 512], fp16)
            nc.scalar.activation(ta[:], tpa[:], Act.Copy)
            tb = wpool.tile([128, 384], fp16)
            nc.gpsimd.memset(tb[96:128, :], 1.0)
            nc.scalar.activation(tb[0:112, :], tpb[:], Act.Copy)
            ta2 = wpool.tile([112, 512], fp16)
            nc.scalar.activation(ta2[:], tpa[:], Act.Copy, scale=LOSCALE)
            tb2 = wpool.tile([128, 384], fp16)
            nc.gpsimd.memset(tb2[96:128, :], LOSCALE)
            nc.scalar.activation(tb2[0:112, :], tpb[:], Act.Copy, scale=LOSCALE)

            # ---- matmul 2: psum2 = y'aug_big @ Abig ----
            p2 = pmm.tile([128, 1024], f32, tag="mm")
            for c in range(NK):
                cc = c % 7
                a, b_ = (ta, tb) if c < 7 else (ta2, tb2)
                if cc < 4:
                    lhsT = a[:, cc * 128:(cc + 1) * 128]
                elif cc < 6:
                    lhsT = b_[0:112, (cc - 4) * 128:(cc - 3) * 128]
                else:
                    lhsT = b_[0:113, 256:384]
                nc.tensor.matmul(p2[:, 0:H0], lhsT, A_sb[c][:, 0:H0],
                                 start=(c == 0), stop=(c == NK - 1))
                nc.tensor.matmul(p2[:, POFF:POFF + H1], lhsT, A_sb[c][:, H0:DA],
                                 start=(c == 0), stop=(c == NK - 1))

            # ---- w = y' * gd_full ; accw ; gd4 ----
            w_t = wpool.tile([128, D], f32)
            accw0 = colp.tile([128, 1], f32)
            accw1 = colp.tile([128, 1], f32)
            nc.vector.scalar_tensor_tensor(
                w_t[:, 0:H0], p2[:, 0:H0], 1.0, yp[:, 0:H0],
                Alu.mult, Alu.mult, accum_out=accw0[:])
            nc.vector.scalar_tensor_tensor(
                w_t[:, H0:D], p2[:, POFF:POFF + (D - H0)], 1.0, yp[:, H0:D],
                Alu.mult, Alu.mult, accum_out=accw1[:])
            gd4 = colp.tile([128, 1], f32)
            nc.vector.tensor_copy(gd4[:], p2[:, POFF + H1 - 1:POFF + H1])

            # ---- softplus terms via Exp+Ln (same ACT table) ----
            # spf_t/spr_t/iw_s/id_s outputs are never read (only the accums
            # matter) — share one rotating scratch tag.
            ez = wpool.tile([128, D], f32)
            spf_t = wpool.tile([128, D], f32, tag="junk", bufs=4)
            spf_c = colp.tile([128, 1], f32)
            nc.scalar.activation(ez[:], d_t[:], Act.Exp, bias=t2n[:], scale=1.0)
            nc.scalar.activation(spf_t[:], ez[:], Act.Ln, bias=1.0, scale=1.0,
                                 accum_out=spf_c[:])
            ez2 = wpool.tile([128, D], f32)
            spr_t = wpool.tile([128, D], f32, tag="junk", bufs=4)
            spr_c = colp.tile([128, 1], f32)
            nc.scalar.activation(ez2[:], w_t[:], Act.Exp, bias=t2n[:], scale=1.0)
            nc.scalar.activation(spr_t[:], ez2[:], Act.Ln, bias=1.0, scale=1.0,
                                 accum_out=spr_c[:])

            # ---- masked sums: iw = sum ind*w ; id = sum ind*d (GPSIMD:
            # off the critical chain, keeps DVE free) ----
            iw_s = wpool.tile([128, D], f32, tag="junk", bufs=4)
            iw_c = colp.tile([128, 1], f32)
            nc.vector.scalar_tensor_tensor(
                iw_s[:], ind[:], 1.0, w_t[:], Alu.mult, Alu.mult,
                accum_out=iw_c[:])
            id_s = wpool.tile([128, D], f32, tag="junk", bufs=4)
            id_c = colp.tile([128, 1], f32)
            nc.vector.scalar_tensor_tensor(
                id_s[:], ind[:], 1.0, d_t[:], Alu.mult, Alu.mult,
                accum_out=id_c[:])

            emit_tail(rs, t, spf_c, spr_c, iw_c, id_c, accw0, accw1,
                      accd0, accd1, gd4, gx4, y_t, yp)


def host_prep(x, W, b, rr, rr_acc, k_iter):
    """Build per-core input maps (numpy, f64 intermediate math)."""
    k = int(k_iter)
    step = MEAN_STEP * (math.cos(math.pi * k / ITER_PER_CYCLE) + 1.0)
    term2 = 1.0 / (2.0 * step)

    W64 = W.astype(np.float64)
    S2 = -0.5 * (W64 + W64.T)
    b2 = b.astype(np.float64) - S2.sum(axis=0)
    A = np.empty((DA, DA), dtype=np.float64)
    A[:D, :D] = S2
    A[D, :D] = b2
    # aug col: psum[:,784] = 0.25*(sum_j gfull_j - b.y) so that
    # gd4-gx4 = 0.25*(sum gd - sum gx) + 0.25*b.(y-y')
    A[:D, D] = 0.25 * (S2.sum(axis=1) - b.astype(np.float64))
    A[D, D] = 0.25 * b2.sum()
    A_hi = A.astype(np.float16)
    A_hi[np.abs(A_hi) < 6.2e-5] = 0      # keep the hi part subnormal-free
    A_lo = ((A - A_hi.astype(np.float64)) / LOSCALE).astype(np.float16)
    Abig = np.ascontiguousarray(np.vstack([A_hi, A_lo]))

    y64 = 1.0 - 2.0 * x.astype(np.float64)
    y = y64.astype(ml_dtypes.bfloat16)
    rr64 = rr.astype(np.float64)
    with np.errstate(divide="ignore"):
        # rr or rr_acc can contain exact 0.0; -inf keeps the same compare
        # semantics as the reference (0 < sigmoid / exp(la) > 0).
        lr2 = (np.log(rr64) - np.log1p(-rr64) + term2).astype(np.float32)
        lacc_full = np.log(rr_acc.astype(np.float64)).astype(np.float32)

    t2neg = np.full((128, 1), -term2, dtype=np.float32)

    in_maps = []
    for c in range(NCORES):
        sl = slice(c * BC, (c + 1) * BC)
        yT = np.empty((DA, BC), dtype=np.float16)
        yT[:D] = y64[sl].T
        yT[D] = 1.0
        in_maps.append({
            "Abig": Abig,
            "yTaug": np.ascontiguousarray(yT),
            "y": np.ascontiguousarray(y[sl]),
            "lr2": np.ascontiguousarray(lr2[sl]),
            "lacc": np.ascontiguousarray(lacc_full[sl].reshape(BC // 128, 128).T),
            "t2neg": t2neg,
        })
    return in_maps


_PROG = {}


def _get_prog():
    if "nc" not in _PROG:
        _PROG["nc"] = build_program()
    return _PROG["nc"]


def kernel(x, W, b, rr, rr_acc, k_iter, _trace=False):
    x = np.asarray(x, dtype=np.float32)
    W = np.asarray(W, dtype=np.float32)
    b = np.asarray(b, dtype=np.float32)
    rr = np.asarray(rr, dtype=np.float32)
    rr_acc = np.asarray(rr_acc, dtype=np.float32)

    in_maps = host_prep(x, W, b, rr, rr_acc, k_iter)
    nc = _get_prog()
    res = run_bass_kernel_spmd(nc, in_maps, list(range(NCORES)), trace=_trace)
    if _trace:
        _PROG["last_result"] = res

    out = np.empty((B, D), dtype=np.float32)
    for c in range(NCORES):
        yn = res.results[c]["out_y"].astype(np.float32)
        out[c * BC:(c + 1) * BC] = (1.0 - yn) * 0.5
    return out

